# revision 34
# baseline (speedup 1.0000x reference)
# Trainium2 Bass kernel for FJSP actor head (gnn_message_passing).
#
# Math (per batch b):
#   job_emb = ops_emb[b, next_op[b], :]                  [50, 128]  (gather)
#   u_j = job_emb @ W1[:128]   v_m = ma_emb[b] @ W1[128:]
#   h1[j,m] = relu(u_j + v_m + b1)            -> 2000 pair columns
#   h2 = relu(h1 @ W2 + b2);  logit = h2 @ W3 + b3
#   noop logit (dummy through the same MLP) is batch-independent -> host.
#
# Device strategy (pure data parallel over batch, 32 batches/core):
#   * Gather reads bf16 rows (ops table pre-cast on host); PE transpose
#     produces jT [E, rows].
#   * ma_emb is pre-transposed to [E, b*40+m] bf16 on host.
#   * The pairwise broadcast u_j + v_m is ONE matmul per batch:
#     lhsT = JV (rows: 50 u's at 0..49, 40 v's at 64..103),
#     rhs = S, a constant 0/1 selection matrix built on host.
#     b1 is folded into the P1 relu drains (per-partition bias).
#   * |W3| is folded into W2/b2 on the host, so the final dot collapses
#     to a signed column-sum: lhsT = sign(W3) [128,1].  The 4 chunk
#     colsums use 4-way column tiling so pairs stream concurrently.
#   * Stage drains (the two relu passes) are the wall: split between
#     DVE (tensor_scalar) and ACT (activation).

import numpy as np
from contextlib import ExitStack

import concourse.bass as bass
import concourse.mybir as mybir
import concourse.tile as tile
from concourse import bacc
from concourse.bass_utils import run_bass_kernel_spmd

BS, N_OPS, N_JOBS, N_MA, E, H = 256, 2000, 50, 40, 128, 128
NCORES = 8
BPC = BS // NCORES            # 32 batches per core
NPAIR = N_JOBS * N_MA         # 2000 pair logits per batch
NPAD = 2048                   # padded pair row (cols 2000:2048 are junk)
PB = 64                       # gather rows reserved per batch (50 real + 14 pad)
NCHUNK = BPC * PB // 128      # 16 gather chunks of 128 rows (2 batches each)
# JV partition layout (K = 104)
R_V0 = 64                     # v_m rows 64..103  (u_j rows at 0..49)
KJV = 104
NCH = 4                       # 512-col chunks per pair row

f32 = mybir.dt.float32
bf16 = mybir.dt.bfloat16

Relu = mybir.ActivationFunctionType.Relu
ADD = mybir.AluOpType.add
MAX = mybir.AluOpType.max


CW = 500                      # live columns per 512-wide chunk block


def _build_smat() -> np.ndarray:
    S = np.zeros((KJV, NPAD), np.float32)
    p = np.arange(NPAIR)
    col = 512 * (p // CW) + p % CW
    for j in range(N_JOBS):
        S[j, col[j * N_MA: (j + 1) * N_MA]] = 1.0
    for m in range(N_MA):
        S[R_V0 + m, col[m: NPAIR: N_MA]] = 1.0
    return S


def _build_module() -> bass.Bass:
    nc = bacc.Bacc("TRN2", target_bir_lowering=False, debug=False)
    gT = nc.dram_tensor("gT", [E, NCHUNK * 128], bf16, kind="ExternalInput")
    maT = nc.dram_tensor("maT", [E, BPC * N_MA], bf16, kind="ExternalInput")
    smat = nc.dram_tensor("smat", [KJV, NPAD], bf16, kind="ExternalInput")
    w1 = nc.dram_tensor("w1", [2 * E, H], bf16, kind="ExternalInput")
    w2 = nc.dram_tensor("w2", [H, H], bf16, kind="ExternalInput")
    sgn = nc.dram_tensor("sgn", [H], bf16, kind="ExternalInput")
    b1v = nc.dram_tensor("b1v", [H], f32, kind="ExternalInput")
    b2v = nc.dram_tensor("b2v", [H], f32, kind="ExternalInput")
    out = nc.dram_tensor("out", [BPC, NPAD], f32, kind="ExternalOutput")

    with tile.TileContext(nc) as tc, ExitStack() as ctx:
        singles = ctx.enter_context(tc.tile_pool(name="singles", bufs=1))

        # ---- input loads, ordered so the gather chain starts ASAP ----
        # Input loads spread across HWDGE rings, ordered by need-time:
        #   sync:   smat blocks (gate the first S-matmul)
        #   vector: pregathered g01 rows (gate the first transpose), maT bulk
        #   scalar: small vectors + wj/wm/w2
        #   tensor: first chunks' maT columns
        # host-pregathered+transposed job rows, chunkwise on the gpsimd
        # ring (chunk 0 gates the whole proj -> jvp -> S pipeline start)
        jt_pool = ctx.enter_context(tc.tile_pool(name="jtp", bufs=16))
        jt_all = []
        for c in range(NCHUNK):
            jT = jt_pool.tile([128, 128], bf16, tag="jt", name=f"jt{c}")
            nc.gpsimd.dma_start(out=jT[:], in_=gT[:, 128 * c:128 * (c + 1)])
            jt_all.append(jT)
        smat_s = singles.tile([KJV, NPAD], bf16)
        for blk in range(NCH):
            nc.sync.dma_start(out=smat_s[:, 512 * blk:512 * (blk + 1)],
                              in_=smat[:, 512 * blk:512 * (blk + 1)])
        maT_s = singles.tile([128, BPC * N_MA], bf16)
        nc.sync.dma_start(out=maT_s[:, 0:4 * N_MA], in_=maT[:, 0:4 * N_MA])
        wj_s = singles.tile([128, H], bf16)
        nc.sync.dma_start(out=wj_s[:], in_=w1[0:E, :])
        wm_s = singles.tile([128, H], bf16)
        nc.sync.dma_start(out=wm_s[:], in_=w1[E:2 * E, :])
        w2_s = singles.tile([128, H], bf16)
        nc.sync.dma_start(out=w2_s[:], in_=w2[:])
        nc.sync.dma_start(out=maT_s[:, 4 * N_MA:], in_=maT[:, 4 * N_MA:])

        sgn_s = singles.tile([128, 1], bf16)
        nc.scalar.dma_start(out=sgn_s[:], in_=sgn[:].rearrange("(p o) -> p o", o=1))
        b1_s = singles.tile([128, 1], f32)
        nc.scalar.dma_start(out=b1_s[:], in_=b1v[:].rearrange("(p o) -> p o", o=1))
        b2_s = singles.tile([128, 1], f32)
        nc.scalar.dma_start(out=b2_s[:], in_=b2v[:].rearrange("(p o) -> p o", o=1))

        # jvp tiles: lhsT for the S-matmul, 2 batches side by side
        jv_pool = ctx.enter_context(tc.tile_pool(name="jvp", bufs=8))

        # psum pools (8 banks total): 2x2 h1 + 2 h2 + 1 pj + 1 lg
        h1_ps = ctx.enter_context(tc.tile_pool(name="h1ps", bufs=2, space="PSUM"))
        h2_ps = ctx.enter_context(tc.tile_pool(name="h2ps", bufs=2, space="PSUM"))
        pj_ps = ctx.enter_context(tc.tile_pool(name="pjps", bufs=1, space="PSUM"))
        lg_ps = ctx.enter_context(tc.tile_pool(name="lgps", bufs=1, space="PSUM"))

        a_pool = ctx.enter_context(tc.tile_pool(name="ap", bufs=6))
        h2s_pool = ctx.enter_context(tc.tile_pool(name="h2s", bufs=6))
        st_pool = ctx.enter_context(tc.tile_pool(name="st", bufs=6))

        # preload the ACT Relu table during the initial DMA window
        relu_warm = singles.tile([1, 2], f32)
        nc.vector.memset(relu_warm[:], 0.0)
        nc.scalar.activation(out=relu_warm[:, 0:1], in_=relu_warm[:, 1:2],
                             func=Relu)

        # PE warm-up during the initial DMA window: junk matmuls (HAM).
        # On the double-buffered h2 pool so they pipeline pairwise and
        # finish before the first transpose/proj needs the PE.
        warm = singles.tile([128, 512], bf16)
        nc.vector.memset(warm[:].bitcast(mybir.dt.uint16), 0)
        for _ in range(2):
            wp = h2_ps.tile([128, 512], f32, tag="h2p", name="warm")
            nc.tensor.matmul(out=wp[:], lhsT=warm[:, 0:128], rhs=warm[:],
                             start=True, stop=True)

        def stage_tp(c):
            """job rows arrive host-pretransposed; just grab a pj bank"""
            pj = pj_ps.tile([128, 2 * 128 + 64], f32, tag="pj", name=f"pj{c}")
            return pj, jt_all[c]

        def stage_proj(c, pj, jT):
            """projection matmuls for chunk c (PE work); junk gather rows
            50:64 land in pj rows 50:64 and are zeroed by S's zero rows"""
            bb = (2 * c, 2 * c + 1)
            for sub in range(2):
                nc.tensor.matmul(out=pj[0:PB, 128 * sub:128 * (sub + 1)],
                                 lhsT=jT[:, sub * PB:(sub + 1) * PB],
                                 rhs=wj_s[:], start=True, stop=True)
                nc.tensor.matmul(out=pj[R_V0:R_V0 + N_MA, 128 * sub:128 * (sub + 1)],
                                 lhsT=maT_s[:, bb[sub] * N_MA:(bb[sub] + 1) * N_MA],
                                 rhs=wm_s[:], start=True, stop=True)

        def stage_drain(c, pj):
            """pj psum -> jvp sbuf drain for chunk c (EW work)"""
            jvp = jv_pool.tile([KJV, 2 * 128], bf16, tag="jv", name=f"jv{c}")
            nc.scalar.copy(out=jvp[0:KJV, :], in_=pj[0:KJV, 0:256])
            return jvp

        # trailing colsum/store state: (batch, H2 tile) pairs emitted two
        # slots late so both batches' P2 drains are long done -> the 8
        # M=1 colsum matmuls sit adjacent in the PE queue and pair up on
        # disjoint col_grps (2 concurrent streams)
        pending = []

        def emit_pending():
            b, H2b = pending.pop(0)
            lg = lg_ps.tile([128, 512], f32, tag="lg", name=f"lg{b}")
            for ci in range(NCH):
                nc.tensor.matmul(out=lg[32 * ci:32 * ci + 1, 0:CW],
                                 lhsT=sgn_s[:],
                                 rhs=H2b[:, 512 * ci:512 * ci + CW],
                                 start=True, stop=True,
                                 tile_position=(0, 32 * ci))
            stg = st_pool.tile([128, 512], f32, tag="st")
            if b % 2 == 0:
                nc.scalar.copy(out=stg[0:97, 0:CW], in_=lg[0:97, 0:CW])
            else:
                nc.vector.tensor_copy(out=stg[0:97, 0:CW], in_=lg[0:97, 0:CW])
            stg4 = stg[:, 0:CW].rearrange("(a b) f -> a b f", b=32)[:, 0:1, :]
            nc.sync.dma_start(
                out=out[b:b + 1, 0:4 * CW].rearrange("o (a f) -> o a f", a=4),
                in_=stg4)

        pj_cur, jT_cur = stage_tp(0)
        stage_proj(0, pj_cur, jT_cur)
        jvp_cur = stage_drain(0, pj_cur)
        for c in range(NCHUNK):
            bb = (2 * c, 2 * c + 1)
            jvp = jvp_cur
            if c + 1 < NCHUNK:
                pj_next, jT_next = stage_tp(c + 1)

            # ---- main pipelines, two batches interleaved ----
            A = [a_pool.tile([128, NPAD], bf16, tag="A", name=f"A{b}") for b in bb]
            H2 = [h2s_pool.tile([128, NPAD], bf16, tag="H2", name=f"H2{b}")
                  for b in bb]
            # S-matmuls (4 same-weight MMs per batch pipeline back-to-back)
            # with wide P1 drains (fused b1 bias) on alternating engines
            for sub in range(2):
                for half in range(2):
                    hp = h1_ps.tile([128, 1024], f32, tag="h1p")
                    for q in range(2):
                        ci = 2 * half + q
                        nc.tensor.matmul(
                            out=hp[:, 512 * q:512 * q + CW],
                            lhsT=jvp[0:KJV, 128 * sub:128 * (sub + 1)],
                            rhs=smat_s[:, 512 * ci:512 * ci + CW],
                            start=True, stop=True)
                    dst = A[sub][:, 1024 * half:1024 * (half + 1)]
                    if (half + sub) % 2 == 0:
                        nc.vector.tensor_scalar(out=dst, in0=hp[:],
                                                scalar1=b1_s[:, 0:1], scalar2=0.0,
                                                op0=ADD, op1=MAX)
                    else:
                        nc.scalar.activation(out=dst, in_=hp[:], func=Relu,
                                             bias=b1_s[:, 0:1])
            if c + 1 < NCHUNK:
                stage_proj(c + 1, pj_next, jT_next)
            # h2 matmuls (both batches back-to-back: one w2 load for 8 MMs)
            # + P2 drains (W3 magnitude pre-folded into w2/b2)
            for sub in range(2):
                b = bb[sub]
                for ci in range(NCH):
                    h2p = h2_ps.tile([128, 512], f32, tag="h2p")
                    nc.tensor.matmul(out=h2p[:, 0:CW], lhsT=w2_s[:],
                                     rhs=A[sub][:, 512 * ci:512 * ci + CW],
                                     start=True, stop=True)
                    dst = H2[sub][:, 512 * ci:512 * ci + CW]
                    if (ci + sub) % 2 == 0:
                        nc.scalar.activation(out=dst, in_=h2p[:, 0:CW],
                                             func=Relu, bias=b2_s[:, 0:1])
                    else:
                        nc.vector.tensor_scalar(out=dst, in0=h2p[:, 0:CW],
                                                scalar1=b2_s[:, 0:1], scalar2=0.0,
                                                op0=ADD, op1=MAX)
                # colsum for the batch finished two slots ago (drain the
                # backlog eagerly on the last chunk to shorten the tail)
                if len(pending) >= (2 if c + 1 < NCHUNK else 1):
                    emit_pending()
                pending.append((b, H2[sub]))
                if sub == 1 and c + 1 < NCHUNK:
                    jvp_cur = stage_drain(c + 1, pj_next)
        while pending:
            emit_pending()

    nc.finalize()
    return nc


_CACHE: dict = {}


def _get_module() -> bass.Bass:
    if "nc" not in _CACHE:
        _CACHE["nc"] = _build_module()
    return _CACHE["nc"]


def _make_in_maps(inputs):
    import ml_dtypes
    bf = ml_dtypes.bfloat16

    ops_emb = np.asarray(inputs["ops_emb"], dtype=np.float32)
    ma_emb = np.asarray(inputs["ma_emb"], dtype=np.float32)
    next_op = np.asarray(inputs["next_op"])
    W1 = np.ascontiguousarray(np.asarray(inputs["W1"], dtype=np.float32).astype(bf))
    b1 = np.ascontiguousarray(np.asarray(inputs["b1"], dtype=np.float32))
    W2 = np.asarray(inputs["W2"], dtype=np.float64)
    b2 = np.asarray(inputs["b2"], dtype=np.float64)
    W3 = np.asarray(inputs["W3"], dtype=np.float64).reshape(-1)   # [H]
    w3a = np.abs(W3)
    W2f = np.ascontiguousarray((W2 * w3a[None, :]).astype(np.float32).astype(bf))
    b2f = np.ascontiguousarray((b2 * w3a).astype(np.float32))
    sgnv = np.ascontiguousarray(np.where(W3 >= 0, 1.0, -1.0).astype(np.float32)
                                .astype(bf))
    smat = _build_smat().astype(bf)

    ops_bf = np.ascontiguousarray(ops_emb.astype(bf))          # [BS, N_OPS, E]
    # maT[core]: [E, BPC*N_MA] with columns b*40+m
    maT = np.ascontiguousarray(
        ma_emb.reshape(NCORES, BPC * N_MA, E).transpose(0, 2, 1).astype(bf))

    in_maps = []
    for core in range(NCORES):
        bsl = slice(core * BPC, (core + 1) * BPC)
        no = np.asarray(next_op[bsl], dtype=np.int64)          # [BPC, 50]
        gidx = np.zeros((BPC, PB), np.int64)
        gidx[:, :N_JOBS] = no + (np.arange(BPC, dtype=np.int64)[:, None] * N_OPS)
        ops_core = ops_bf[bsl].reshape(BPC * N_OPS, E)
        gTc = np.ascontiguousarray(ops_core[gidx.ravel()].T)   # [E, 2048]
        in_maps.append({
            "gT": gTc,
            "maT": maT[core],
            "smat": smat,
            "w1": W1, "w2": W2f, "sgn": sgnv,
            "b1v": b1, "b2v": b2f,
        })
    return in_maps


def _host_noop(inputs) -> np.ndarray:
    dummy = np.asarray(inputs["dummy"], dtype=np.float64)
    W1 = np.asarray(inputs["W1"], dtype=np.float64)
    b1 = np.asarray(inputs["b1"], dtype=np.float64)
    W2 = np.asarray(inputs["W2"], dtype=np.float64)
    b2 = np.asarray(inputs["b2"], dtype=np.float64)
    W3 = np.asarray(inputs["W3"], dtype=np.float64)
    b3 = np.asarray(inputs["b3"], dtype=np.float64)
    d1 = np.maximum(dummy @ W1 + b1, 0.0)
    d2 = np.maximum(d1 @ W2 + b2, 0.0)
    return (d2 @ W3 + b3).astype(np.float32)  # [1]


def _run(inputs, trace=False, **kw):
    action_mask = np.asarray(inputs["action_mask"])
    b3 = np.asarray(inputs["b3"], dtype=np.float32)
    nc = _get_module()
    in_maps = _make_in_maps(inputs)
    res = run_bass_kernel_spmd(nc, in_maps, core_ids=list(range(NCORES)),
                               trace=trace, **kw)
    logits = np.empty((BS, N_JOBS * N_MA + 1), np.float32)
    pair = np.concatenate([r["out"][:, :NPAIR] for r in res.results], axis=0)
    logits[:, 1:] = pair + b3.reshape(-1)[0]
    logits[:, 0] = _host_noop(inputs)[0]
    return (logits, action_mask), res


def kernel(**inputs):
    out, _ = _run(inputs)
    return out
